# revision 22
# baseline (speedup 1.0000x reference)
"""Trainium2 Bass kernel for nn_CompSAE (topk_masking, memory-bound).

Math (after host-side folding of the seq_len-1 attention + biases):
    f  = relu(x @ W1 + b1_eff)            # [N, 256],  W1 = W_enc_f
    c  = relu(f @ W2 + b2)                # [N, 128],  W2 = W_enc_c
    bn = relu(c @ Wb + bb)                # [N, 32],   Wb = W_v.T @ W_out.T @ W_bottleneck
    y  = bn @ W_dec + f @ W_res + b_dec   # [N, 2048]

Sharding: pure data-parallel over the token axis N=131072 across 8 cores
(16384 tokens/core). All weights replicated.

All matmuls are fp16 with fp32 PSUM accumulation (~6e-4 scale-relative error;
fp8 DoubleRow measured 2.6e-2 even in its mildest form — over the 2e-2 gate).
At f16 the PE floor is 83 streamed columns/token = 566us/core, so the kernel
is built to keep TensorE dense:
  - host packs x and the weights into partition-major tiles whose DMA is
    contiguous 8-16KB per-partition lines (line-rate packets),
  - y is written f16 in [supertile, th, token128, d] order (contiguous per-th
    DMAs; host unpack is a reshape + f32 upcast),
  - mm4 groups 4 matmuls per stationary (fT/bnT slices), and mm2/mm3 are
    interleaved into the mm4 stream so ACT latency never stalls the PE,
  - bnT lives in two persistent tiles (zero rows + b_dec ones-row written
    once); only relu rows are rewritten per supertile.
"""

import os
import numpy as np

N_TOK, D_IN, D_F, D_C, K_BN = 131072, 2048, 256, 128, 32
N_CORES = 8
SHARD = N_TOK // N_CORES          # 16384 tokens per core
TOK = 512                         # supertile tokens
N_SUPER = SHARD // TOK            # 32 supertiles
KC = D_IN // 128                  # 16 contraction chunks for mm1

_CACHE = {}


def _build_nc():
    import concourse.tile as tile
    from concourse import bacc, mybir
    from concourse.bass import ts

    f32 = mybir.dt.float32
    f16 = mybir.dt.float16
    Relu = mybir.ActivationFunctionType.Relu

    nc = bacc.Bacc(None, target_bir_lowering=False)

    xP_d = nc.dram_tensor("xP", [N_SUPER, 2, 128, KC // 2, TOK], f16,
                          kind="ExternalInput")
    # w1 split into m-halves so the m=0 half (all mm1 needs first) is a single
    # 512KB contiguous load racing ahead of the x prefetch on the sync queue.
    w1_d = nc.dram_tensor("w1", [2, 128, KC, 128], f16, kind="ExternalInput")
    w2_d = nc.dram_tensor("w2", [128, 2, D_C], f16, kind="ExternalInput")
    # wb zero-padded to [128, 128] (cols 32..127 zero) and wdec_aug zero-padded
    # to [128, 2048] (rows 33..127 zero): uniform 128-row stationaries pipeline
    # on the PE; the K=33 form stalled ~250ns per matmul on reconfig.
    wb_d = nc.dram_tensor("wb", [D_C, 128], f16, kind="ExternalInput")
    wres_d = nc.dram_tensor("wres", [128, 2, D_IN], f16, kind="ExternalInput")
    wdec_d = nc.dram_tensor("wdec", [128, D_IN], f16, kind="ExternalInput")
    b1_d = nc.dram_tensor("b1", [128, 2], f32, kind="ExternalInput")
    b2_d = nc.dram_tensor("b2", [128, 1], f32, kind="ExternalInput")
    bb_d = nc.dram_tensor("bb", [K_BN, 1], f32, kind="ExternalInput")
    y_d = nc.dram_tensor("y", [N_SUPER, TOK // 128, 128, D_IN], f16,
                         kind="ExternalOutput")

    with tile.TileContext(nc) as tc:
        with (
            tc.tile_pool(name="const", bufs=1) as const,
            tc.tile_pool(name="xtp", bufs=3) as xtp,
            tc.tile_pool(name="fp", bufs=2) as fp,
            tc.tile_pool(name="cp", bufs=2) as cp,
            tc.tile_pool(name="yp", bufs=2) as yp,
            tc.tile_pool(name="fps", bufs=2, space="PSUM") as fps,
            tc.tile_pool(name="sps", bufs=1, space="PSUM") as sps,
            tc.tile_pool(name="yps", bufs=1, space="PSUM") as yps,
        ):
            # DMA ordering at startup: sync queue carries w1[m0] then the x
            # tiles (FIFO -> w1[m0] + x0 arrive first); the scalar queue
            # carries w1[m1] and the mm4 weights, whose bytes interleave with
            # the sync queue's and land before their first use (~16-20us in).
            w1_sb = const.tile([128, 2, KC, 128], f16)
            nc.sync.dma_start(w1_sb[:, 0], w1_d[0])
            nc.scalar.dma_start(w1_sb[:, 1], w1_d[1])
            # wres loads on the sync queue right behind x tile 0 (below);
            # it arrives just before supertile 0's mm4 needs it.
            wres_sb = const.tile([128, 2, D_IN], f16)
            wdec_sb = const.tile([128, D_IN], f16)
            nc.scalar.dma_start(wdec_sb[:], wdec_d[:])
            w2_sb = const.tile([128, 2, D_C], f16)
            nc.scalar.dma_start(w2_sb[:], w2_d[:])
            wb_sb = const.tile([D_C, 128], f16)
            nc.scalar.dma_start(wb_sb[:], wb_d[:])
            b1_sb = const.tile([128, 2], f32)
            nc.scalar.dma_start(b1_sb[:], b1_d[:])
            b2_sb = const.tile([128, 1], f32)
            nc.scalar.dma_start(b2_sb[:], b2_d[:])
            bb_sb = const.tile([K_BN, 1], f32)
            nc.scalar.dma_start(bb_sb[:], bb_d[:])

            # PE HAM warm-up: ~3.4us of dummy matmuls (uninitialized operands,
            # dummy psum slot) run while the first DMAs are in flight, so the
            # first real matmul starts at 2.4GHz instead of 1.2.
            warm_sb = const.tile([128, 512], f16)
            nc.gpsimd.memset(warm_sb[:], 0.0)
            warm_ps = sps.tile([128, TOK], f32, tag="small", name="warm_ps")
            for i in range(16):
                nc.tensor.matmul(warm_ps[:], warm_sb[:, 0:128], warm_sb[:],
                                 start=True, stop=True)

            # persistent bnT tiles: rows 0..31 = relu'd bottleneck (rewritten
            # each supertile), row 32 = ones (b_dec row of wdec_aug), rows
            # 33..127 = zero. Uniform 128-row stationary for mm4.
            bnTs = []
            for i in range(2):
                bnT_i = const.tile([128, TOK], f16, name=f"bnT{i}")
                nc.gpsimd.memset(bnT_i[:], 0.0)
                nc.gpsimd.memset(bnT_i[K_BN:K_BN + 1, :], 1.0)
                bnTs.append(bnT_i)

            for t in range(N_SUPER):
                xT = xtp.tile([128, KC, TOK], f16)
                if t == 0:
                    # quarter-granularity so mm1's c-loop chases the DMA
                    for h in range(2):
                        for q in range(2):
                            nc.sync.dma_start(
                                xT[:, h * (KC // 2) + q * (KC // 4):
                                   h * (KC // 2) + (q + 1) * (KC // 4), :],
                                xP_d[t, h][:, ts(q, KC // 4), :],
                            )
                    nc.sync.dma_start(wres_sb[:], wres_d[:])
                else:
                    for h in range(2):
                        nc.sync.dma_start(xT[:, ts(h, KC // 2), :], xP_d[t, h])

                fT = fp.tile([128, 2, TOK], f16)
                cps = sps.tile([128, TOK], f32, tag="small")
                bnT = bnTs[t % 2]

                # mm1 m=0
                ps0 = fps.tile([128, TOK], f32, tag="mm1ps")
                for c in range(KC):
                    nc.tensor.matmul(
                        ps0[:], w1_sb[:, 0, c, :], xT[:, c, :],
                        start=(c == 0), stop=(c == KC - 1),
                    )
                nc.scalar.activation(fT[:, 0, :], ps0[:], Relu, bias=b1_sb[:, 0:1])

                # mm1 m=1, with mm2's first half slotted in once fT[0] is ready
                ps1 = fps.tile([128, TOK], f32, tag="mm1ps")
                for c in range(KC):
                    nc.tensor.matmul(
                        ps1[:], w1_sb[:, 1, c, :], xT[:, c, :],
                        start=(c == 0), stop=(c == KC - 1),
                    )
                    if c == 5:
                        nc.tensor.matmul(
                            cps[:], w2_sb[:, 0, :], fT[:, 0, :],
                            start=True, stop=False,
                        )
                nc.scalar.activation(fT[:, 1, :], ps1[:], Relu, bias=b1_sb[:, 1:2])

                # mm4 th=0 f-part, with mm2's second half and mm3 slotted in so
                # the ACT chain (cT -> bnT) completes before th0's bn matmuls.
                cT = cp.tile([128, TOK], f16)
                bps = sps.tile([128, TOK], f32, tag="small")
                y_sb = yp.tile([128, TOK // 128, D_IN], f16)

                for th in range(TOK // 128):
                    # rotate through 5 psum banks so a new th-group's first
                    # matmul never waits on the previous group's copy-out
                    pss = [yps.tile([128, 512], f32,
                                    name=f"ypsb{(t * 16 + th * 4 + n) % 5}")
                           for n in range(4)]
                    for n in range(4):
                        nc.tensor.matmul(
                            pss[n][:], fT[:, 0, ts(th, 128)], wres_sb[:, 0, ts(n, 512)],
                            start=True, stop=False,
                        )
                    # mm2's second half, then the cT -> mm3 -> bnT chain split
                    # into token-column halves: the first half is ready before
                    # th0's bn matmuls (which only read bnT cols 0:128), the
                    # second half has multi-us slack (first read at th2).
                    for n in range(4):
                        nc.tensor.matmul(
                            pss[n][:], fT[:, 1, ts(th, 128)], wres_sb[:, 1, ts(n, 512)],
                            start=False, stop=False,
                        )
                        if th == 0 and n == 0:
                            nc.tensor.matmul(
                                cps[:], w2_sb[:, 1, :], fT[:, 1, :],
                                start=False, stop=True,
                            )
                            nc.scalar.activation(cT[:, 0:256], cps[:, 0:256],
                                                 Relu, bias=b2_sb[:])
                        if th == 0 and n == 2:
                            nc.tensor.matmul(bps[:, 0:256], wb_sb[:], cT[:, 0:256])
                            nc.scalar.activation(
                                bnT[0:K_BN, 0:256], bps[0:K_BN, 0:256],
                                Relu, bias=bb_sb[:]
                            )
                    if th == 0:
                        nc.scalar.activation(cT[:, 256:512], cps[:, 256:512],
                                             Relu, bias=b2_sb[:])
                        nc.tensor.matmul(bps[:, 256:512], wb_sb[:], cT[:, 256:512])
                        nc.scalar.activation(
                            bnT[0:K_BN, 256:512], bps[0:K_BN, 256:512],
                            Relu, bias=bb_sb[:]
                        )
                    for n in range(4):
                        nc.tensor.matmul(
                            pss[n][:], bnT[:, ts(th, 128)], wdec_sb[:, ts(n, 512)],
                            start=False, stop=True,
                        )
                    for n in range(4):
                        # DVE-heavy split: ACT also runs the activation chain
                        # and issues the y DMAs, DVE is otherwise mostly idle
                        if n == 0:
                            nc.scalar.copy(out=y_sb[:, th, ts(n, 512)], in_=pss[n][:])
                        else:
                            nc.vector.tensor_copy(out=y_sb[:, th, ts(n, 512)], in_=pss[n][:])
                    nc.scalar.dma_start(y_d[t, th], y_sb[:, th, :])

    nc.compile()
    return nc


def _fold_weights(inputs):
    f64 = np.float64
    W1 = np.asarray(inputs["W_enc_f"], np.float32)
    W2 = np.asarray(inputs["W_enc_c"], np.float32)
    W_v = np.asarray(inputs["W_v"], f64)
    b_v = np.asarray(inputs["b_v"], f64)
    W_out = np.asarray(inputs["W_out"], f64)
    b_out = np.asarray(inputs["b_out"], f64)
    W_bn = np.asarray(inputs["W_bottleneck"], f64)
    W_dec = np.asarray(inputs["W_dec"], np.float32)
    b_dec = np.asarray(inputs["b_dec"], np.float32)
    W_res = np.asarray(inputs["W_res"], np.float32)
    b1_eff = (np.asarray(inputs["b_enc_f"], f64)
              - np.asarray(inputs["b_dec"], f64) @ np.asarray(inputs["W_enc_f"], f64))
    Wb = (W_v.T @ W_out.T) @ W_bn                      # [128, 32]
    bb = (b_v @ W_out.T + b_out) @ W_bn                # [32]
    wdec_aug = np.vstack([W_dec, b_dec[None, :]])      # [33, 2048]

    # partition-major packing: [128, a, n] with w[p, a, n] = W[a*128 + p, n];
    # w1 additionally m-major: w1p[m, p, c, j] = W1[c*128 + p, m*128 + j]
    w1p = np.ascontiguousarray(
        W1.reshape(KC, 128, 2, 128).transpose(2, 1, 0, 3).astype(np.float16))
    w2p = np.ascontiguousarray(
        W2.reshape(2, 128, D_C).transpose(1, 0, 2).astype(np.float16))
    wresp = np.ascontiguousarray(
        W_res.reshape(2, 128, D_IN).transpose(1, 0, 2).astype(np.float16))

    return {
        "w1": w1p,
        "w2": w2p,
        "wb": np.ascontiguousarray(
            np.pad(Wb.astype(np.float16), ((0, 0), (0, 128 - K_BN)))),
        "wres": wresp,
        "wdec": np.ascontiguousarray(
            np.pad(wdec_aug.astype(np.float16), ((0, 128 - K_BN - 1), (0, 0)))),
        "b1": np.ascontiguousarray(b1_eff.astype(np.float32).reshape(2, 128).T),
        "b2": np.ascontiguousarray(np.asarray(inputs["b_enc_c"], np.float32).reshape(128, 1)),
        "bb": np.ascontiguousarray(bb.astype(np.float32).reshape(K_BN, 1)),
    }


def _pack_x(x_core):
    """[SHARD, D_IN] f32 -> [N_SUPER, 2, 128, KC//2, TOK] f16 with
    packed[s, h, p, c, t] = x[s*TOK + t, (h*KC//2 + c)*128 + p]."""
    v = x_core.astype(np.float16).reshape(N_SUPER, TOK, 2, KC // 2, 128)
    return np.ascontiguousarray(v.transpose(0, 2, 4, 3, 1))


def kernel(**inputs) -> np.ndarray:
    from concourse.bass_utils import run_bass_kernel_spmd

    if "nc" not in _CACHE:
        _CACHE["nc"] = _build_nc()
    nc = _CACHE["nc"]

    x = np.asarray(inputs["acts"], np.float32)
    weights = _fold_weights(inputs)

    in_maps = []
    for i in range(N_CORES):
        m = {"xP": _pack_x(x[i * SHARD:(i + 1) * SHARD])}
        m.update(weights)
        in_maps.append(m)

    trace = bool(os.environ.get("BASS_KERNEL_TRACE"))
    res = run_bass_kernel_spmd(
        nc, in_maps, core_ids=list(range(N_CORES)), trace=trace,
    )
    _CACHE["last_result"] = res
    # y[t, a, p, d] is already token-major: token = t*512 + a*128 + p
    return np.concatenate(
        [np.asarray(res.results[i]["y"]).reshape(SHARD, D_IN).astype(np.float32)
         for i in range(N_CORES)],
        axis=0,
    )
